# revision 3
# baseline (speedup 1.0000x reference)
"""Multi-head causal attention (B=2, S=2048, E=1024, H=16, D=64) on 8 TRN2
NeuronCores.

Sharding (data + tensor parallel, Megatron-style):
  core c -> batch b = c // 4, head group g = c % 4 (4 heads, e' = 256 cols).
  Wq/Wk/Wv column-sharded ([256, 1024] slices), Wo row-sharded
  ([1024, 256] slice); each core produces a partial output [2048, 1024]
  (f16) which the host sums per batch group (the Megatron all-reduce) and
  adds bo.

Per-core device kernel (matmul operands fp16, accumulate fp32 in PSUM).
v2 schedule: fine-grained deadline-driven weave.
  - DMA order puts the exp-critical path first (wk, xk0, wq, xq3) so the
    ACT exp stream starts ~21us in; everything else chases.
  - logits pairs (2 heads on PE row-tiles 0-63/64-127) run concurrently;
    attnV (V' ones column -> PSUM row 64 = denominator) lags the exp
    stream; projection/O-proj matmuls are pumped one-at-a-time from a
    deadline-ordered filler queue between attention slots, so the PE
    never idles multi-us on phase boundaries and ACT never starves.
  - normalize: accs copied to SBUF (frees PSUM), reciprocal of the
    denominator row in-place on DVE ([1, 512] rows), gpsimd
    partition-broadcast, DVE multiply into valsT. No DMA transposes.
  - last q-window normalizes directly from PSUM and the tail is just
    4 O-proj tiles + casts + out-DMA.
"""
import sys
import os

sys.path.insert(0, "/opt/trn_rl_repo")

import numpy as np
from contextlib import ExitStack

import concourse.bass as bass  # noqa: E402
import concourse.mybir as mybir  # noqa: E402
import concourse.tile as tile  # noqa: E402
from concourse import bacc, bass_utils  # noqa: E402

bass_utils.upload_artifacts = lambda d: f"local:{d}"

B, S, E, H, D = 2, 2048, 1024, 16, 64
NCORES = 8
EL = 256  # e' columns per core (4 heads)
F32 = mybir.dt.float32
F16 = mybir.dt.float16
AF = mybir.ActivationFunctionType

_CACHE = {}

LAG3 = 6   # attnV lag during (qt=3, c=0): waits for xv DMA + vproj
LAG = 2    # attnV lag elsewhere


def _build():
    nc = bacc.Bacc("TRN2", target_bir_lowering=False, debug=False)

    # x tensors host-pre-blocked: [tb, p, k*512 + m]
    xq_d = nc.dram_tensor("xqB", [4, 128, 8 * 512], F16, kind="ExternalInput")
    xk_d = nc.dram_tensor("xkB", [4, 128, 8 * 512], F16, kind="ExternalInput")
    xv_d = nc.dram_tensor("xvB", [4, 128, 8 * 512], F16, kind="ExternalInput")
    wq_d = nc.dram_tensor("wqT", [E, EL], F16, kind="ExternalInput")
    wk_d = nc.dram_tensor("wkT", [E, EL], F16, kind="ExternalInput")
    wv_d = nc.dram_tensor("wvT", [E, EL], F16, kind="ExternalInput")
    wo_d = nc.dram_tensor("woT", [EL, E], F16, kind="ExternalInput")
    bq_d = nc.dram_tensor("bq", [EL], F32, kind="ExternalInput")
    bk_d = nc.dram_tensor("bk", [EL], F32, kind="ExternalInput")
    bv_d = nc.dram_tensor("bv", [EL], F32, kind="ExternalInput")
    vones_d = nc.dram_tensor("vones", [128, 16, 4, 1], F16, kind="ExternalInput")
    mask_d = nc.dram_tensor("masks", [128, 2, 128], F16, kind="ExternalInput")
    out_d = nc.dram_tensor("out", [S, E], F16, kind="ExternalOutput")

    with tile.TileContext(nc) as tc, ExitStack() as ctx:
        cpool = ctx.enter_context(tc.tile_pool(name="const", bufs=1))
        psp = ctx.enter_context(tc.tile_pool(name="psp", bufs=2, space="PSUM"))
        expp = ctx.enter_context(tc.tile_pool(name="expp", bufs=10))
        opool = ctx.enter_context(tc.tile_pool(name="op", bufs=2))
        smp = ctx.enter_context(tc.tile_pool(name="smp", bufs=2))

        xk = cpool.tile([128, 8, S], F16, tag="xk")
        xq = cpool.tile([128, 8, S], F16, tag="xq")
        xv = cpool.tile([128, 8, S], F16, tag="xv")

        def xblock(x_t, x_d, tb):
            nc.sync.dma_start(
                x_t[:, :, tb * 512:(tb + 1) * 512],
                x_d.ap()[tb].rearrange("p (k m) -> p k m", k=8))

        # ---- DMA issue order == arrival order (single HWDGE ring).
        # Critical path to first exp: wk, xk0, wq, xq3. Then k/v/q blocks
        # by first-use deadline.
        wk = cpool.tile([128, 8, EL], F16, tag="wk")
        nc.sync.dma_start(wk[:], wk_d.ap().rearrange("(k p) m -> p k m", p=128))
        xblock(xk, xk_d, 0)
        wq = cpool.tile([128, 8, EL], F16, tag="wq")
        nc.sync.dma_start(wq[:], wq_d.ap().rearrange("(k p) m -> p k m", p=128))
        xblock(xq, xq_d, 3)
        bkt = cpool.tile([128, 2], F32, tag="bkt")
        nc.sync.dma_start(bkt[:], bk_d.ap().rearrange("(c p) -> p c", p=128))
        bqt = cpool.tile([128, 2], F32, tag="bqt")
        nc.sync.dma_start(bqt[:], bq_d.ap().rearrange("(c p) -> p c", p=128))
        mk2 = cpool.tile([128, 2, 128], F16, tag="mk2")
        nc.sync.dma_start(mk2[:], mask_d.ap())
        xblock(xk, xk_d, 1)
        wv = cpool.tile([128, 8, EL], F16, tag="wv")
        nc.sync.dma_start(wv[:], wv_d.ap().rearrange("(k p) m -> p k m", p=128))
        bvr = cpool.tile([1, EL], F32, tag="bvr")
        nc.sync.dma_start(bvr[:], bv_d.ap().rearrange("(p m) -> p m", p=1))
        bvb = cpool.tile([128, EL], F32, tag="bvb")
        nc.gpsimd.partition_broadcast(bvb[:], bvr[:])
        VP = cpool.tile([128, 16, 4 * 66], F16, tag="VP")  # 66: 4B-aligned
        nc.sync.dma_start(
            VP[:].rearrange("p k (h x) -> p k h x", h=4)[:, :, :, 64:65],
            vones_d.ap())
        xblock(xv, xv_d, 0)
        xblock(xk, xk_d, 2)
        xblock(xv, xv_d, 1)
        xblock(xk, xk_d, 3)
        xblock(xq, xq_d, 2)
        xblock(xv, xv_d, 2)
        xblock(xv, xv_d, 3)
        wo = cpool.tile([128, 2, E], F16, tag="wo")
        nc.sync.dma_start(wo[:], wo_d.ap().rearrange("(c p) m -> p c m", p=128))
        xblock(xq, xq_d, 1)
        xblock(xq, xq_d, 0)

        KT = cpool.tile([128, 2, S], F16, tag="KT")
        QT = cpool.tile([128, 2, S], F16, tag="QT")
        valsT = cpool.tile([128, 2, S], F16, tag="valsT")

        # ---- filler chains: generators yielding after each matmul ----
        def g_kproj(tb, tag="ops"):
            for c in range(2):
                ps = psp.tile([128, 512], F32, tag=tag,
                              bufs=2, name=f"kps{tb}_{c}")
                for k in range(8):
                    nc.tensor.matmul(
                        ps[:],
                        lhsT=wk[:, k, c * 128:(c + 1) * 128],
                        rhs=xk[:, k, tb * 512:(tb + 1) * 512],
                        start=(k == 0), stop=(k == 7))
                    yield
                nc.vector.tensor_scalar_add(
                    KT[:, c, tb * 512:(tb + 1) * 512], ps[:], bkt[:, c:c + 1])

        def g_qproj(tt, tag="ops"):
            for c in range(2):
                ps = psp.tile([128, 512], F32, tag=tag,
                              bufs=2, name=f"qps{tt}_{c}")
                for k in range(8):
                    nc.tensor.matmul(
                        ps[:],
                        lhsT=wq[:, k, c * 128:(c + 1) * 128],
                        rhs=xq[:, k, tt * 512:(tt + 1) * 512],
                        start=(k == 0), stop=(k == 7))
                    yield
                nc.vector.tensor_scalar_add(
                    QT[:, c, tt * 512:(tt + 1) * 512], ps[:], bqt[:, c:c + 1])

        def g_vproj(t3):
            ps = psp.tile([128, EL], F32, tag="ops", bufs=2, name=f"vps{t3}")
            for k in range(8):
                nc.tensor.matmul(
                    ps[:],
                    lhsT=xv[:, k, t3 * 128:(t3 + 1) * 128],
                    rhs=wv[:, k, :],
                    start=(k == 0), stop=(k == 7))
                yield
            nc.vector.tensor_add(
                VP[:, t3, :].rearrange("p (h x) -> p h x", h=4)[:, :, 0:64],
                ps[:].rearrange("p (h x) -> p h x", h=4),
                bvb[:].rearrange("p (h x) -> p h x", h=4))

        def g_oproj(tp):
            # one chain covers tile pair (2*tp, 2*tp+1); single out DMA
            ot = opool.tile([128, 2, 2, 512], F16, tag="ot", name=f"ot{tp}")
            for ti in range(2):
                tt = 2 * tp + ti
                for eo in range(2):
                    ps = psp.tile([128, 512], F32, tag="ops", bufs=2,
                                  name=f"ops{tt}_{eo}")
                    for c in range(2):
                        nc.tensor.matmul(
                            ps[:],
                            lhsT=valsT[:, c, tt * 128:(tt + 1) * 128],
                            rhs=wo[:, c, eo * 512:(eo + 1) * 512],
                            start=(c == 0), stop=(c == 1))
                        yield
                    nc.vector.tensor_copy(ot[:, ti, eo, :], ps[:])
            nc.gpsimd.dma_start(
                out_d.ap()[tp * 256:(tp + 1) * 256, :].rearrange(
                    "(a p) m -> p a m", p=128),
                ot[:].rearrange("p a b m -> p a (b m)"))

        chains = {}
        order = []

        def add_chain(name, gen):
            chains[name] = gen
            order.append(name)

        def pump(n):
            # strictly front-of-queue: at most one chain mid-flight, so the
            # shared "ops" PSUM tag never has two incomplete accumulations.
            done = 0
            while done < n and order:
                name = order[0]
                try:
                    next(chains[name])
                    done += 1
                except StopIteration:
                    del chains[name]
                    order.pop(0)

        def ensure(name):
            while name in chains:
                pump(1)

        # pre-exp critical path: K block 0 + Q tile 3 (first lg needs only
        # these); kproj(1..3)/vproj/qproj chase as filler.
        for _ in g_kproj(0, tag="lg"):
            pass
        for _ in g_qproj(3, tag="lg"):
            pass

        # deadline-ordered filler for qt=3 c=0: kproj by lg-tile need,
        # vproj by attnV(lag) need.
        add_chain("k1", g_kproj(1))
        add_chain("v0", g_vproj(0))
        add_chain("v1", g_vproj(1))
        add_chain("k2", g_kproj(2))
        add_chain("v2", g_vproj(2))
        add_chain("v3", g_vproj(3))
        add_chain("v4", g_vproj(4))
        add_chain("v5", g_vproj(5))
        add_chain("k3", g_kproj(3))
        for t in range(6, 16):
            add_chain(f"v{t}", g_vproj(t))

        # filler arriving later (registered at the phase that precedes
        # their deadline): qproj(2) during qt3c1, oproj(12..15)+qproj(1)
        # during qt2, oproj(8..11)+qproj(0) during qt1, oproj(4..7) during
        # qt0, oproj(0..3) at the tail.
        late = {
            (3, 1): [("q2", lambda: g_qproj(2))],
            (2, 0): [("o6", lambda: g_oproj(6)), ("o7", lambda: g_oproj(7)),
                     ("q1", lambda: g_qproj(1))],
            (1, 0): [("o4", lambda: g_oproj(4)), ("o5", lambda: g_oproj(5)),
                     ("q0", lambda: g_qproj(0))],
            (0, 0): [("o2", lambda: g_oproj(2)), ("o3", lambda: g_oproj(3))],
        }
        # per-(qt, c) pump budget (filler matmuls issued after each lg_exp)
        budget = {
            (3, 0): 12, (3, 1): 3,
            (2, 0): 4, (2, 1): 2,
            (1, 0): 5, (1, 1): 2,
            (0, 0): 6, (0, 1): 2,
        }

        for qt in range(3, -1, -1):
            nkt = 4 * qt + 4
            lag = LAG3 if qt == 3 else LAG
            accs = {}
            for c in range(2):
                for hh in range(2):
                    accs[(c, hh)] = psp.tile([65, 512], F32, tag="acc",
                                             bufs=2, name=f"acc{qt}_{c}_{hh}")
            exs = {}

            def lg_exp(c, kt):
                dd = kt * 128 - qt * 512
                s = max(dd, 0)
                lg = psp.tile([128, 2, 512], F32, tag="lg", bufs=2,
                              name=f"lg{qt}_{c}_{kt}")
                for hh in range(2):
                    nc.tensor.matmul(
                        lg[:, hh, s:512],
                        lhsT=KT[hh * 64:(hh + 1) * 64, c,
                                kt * 128:(kt + 1) * 128],
                        rhs=QT[hh * 64:(hh + 1) * 64, c,
                               qt * 512 + s:(qt + 1) * 512],
                        start=True, stop=True)
                ex = expp.tile([128, 2, 512], F16, tag="ex",
                               name=f"ex{qt}_{c}_{kt}")
                nc.scalar.activation(ex[:, :, s:512], lg[:, :, s:512], AF.Exp,
                                     scale=0.125)
                if dd >= 0:  # diagonal tile: lower-tri mask on first 128 cols
                    nc.vector.tensor_mul(ex[:, :, s:s + 128],
                                         ex[:, :, s:s + 128], mk2[:])
                exs[(c, kt)] = ex

            def attn_v(c, kt):
                ex = exs.pop((c, kt))
                s = max(kt * 128 - qt * 512, 0)
                for hh in range(2):
                    h = 2 * c + hh
                    nc.tensor.matmul(
                        accs[(c, hh)][:, s:512],
                        lhsT=VP[:, kt, h * 66:h * 66 + 65],
                        rhs=ex[:, hh, s:512],
                        start=(kt == 0), stop=(kt == nkt - 1),
                        skip_group_check=True)

            for c in range(2):
                for name, mk in late.get((qt, c), []):
                    add_chain(name, mk())
                bgt = budget[(qt, c)]
                for kt in range(nkt):
                    if qt == 3 and c == 0 and kt >= 4 and kt % 4 == 0:
                        ensure(f"k{kt // 4}")
                    lg_exp(c, kt)
                    pump(bgt)
                    if kt >= lag:
                        if qt == 3 and c == 0:
                            ensure(f"v{kt - lag}")
                        attn_v(c, kt - lag)
                for kt in range(nkt - lag, nkt):
                    if qt == 3 and c == 0:
                        ensure(f"v{kt}")
                    attn_v(c, kt)

                # ---- normalize chunk c ----
                # qt>0: acc -> SBUF copy (frees PSUM), reciprocal of the
                # denominator row in place on DVE, gpsimd broadcast, DVE
                # multiply into valsT. qt==0: straight from PSUM (no reuse
                # pressure at the end).
                if qt > 0:
                    svs = {}
                    for hh in range(2):
                        sv = smp.tile([65, 512], F32, tag="sv", bufs=4,
                                      name=f"sv{qt}_{c}_{hh}")
                        nc.vector.tensor_copy(sv[:], accs[(c, hh)][:])
                        svs[hh] = sv
                    den = {hh: svs[hh][64:65, :] for hh in range(2)}
                    body = {hh: svs[hh][0:64, :] for hh in range(2)}
                else:
                    den = {hh: accs[(c, hh)][64:65, :] for hh in range(2)}
                    body = {hh: accs[(c, hh)][0:64, :] for hh in range(2)}
                rrow = smp.tile([1, 2, 512], F32, tag="rrow", bufs=2,
                                name=f"rr{qt}_{c}")
                for hh in range(2):
                    nc.vector.reciprocal(rrow[0:1, hh, :], den[hh])
                for hh in range(2):
                    bc = smp.tile([128, 512], F32, tag="bc", bufs=4,
                                  name=f"bc{qt}_{c}_{hh}")
                    nc.gpsimd.partition_broadcast(bc[0:64, :], rrow[0:1, hh, :])
                    nc.vector.tensor_mul(
                        valsT[hh * 64:(hh + 1) * 64, c,
                              qt * 512:(qt + 1) * 512],
                        body[hh], bc[0:64, :])

        # tail: first q-window's O-projection pairs
        for name in ("o0", "o1"):
            add_chain(name, g_oproj(int(name[1])))
        while order:
            pump(1000)

    nc.compile()
    return nc


def get_nc():
    if "nc" not in _CACHE:
        _CACHE["nc"] = _build()
    return _CACHE["nc"]


def _masks():
    i = np.arange(128)[:, None]
    j = np.arange(128)[None, :]
    m = (i <= j).astype(np.float16)  # within-window causal: keep k <= q
    return np.broadcast_to(m[:, None, :], (128, 2, 128)).copy()


def _xblocks(x):
    # [S, E] f32 -> [4, 128, 8*512] f16: blk[tb, p, k*512+m] = x[tb*512+m, k*128+p]
    xT = np.ascontiguousarray(x.T).astype(np.float16)  # [E, S]
    return np.ascontiguousarray(
        xT.reshape(8, 128, 4, 512).transpose(2, 1, 0, 3).reshape(4, 128, 4096))


def make_in_maps(query, key, value, Wq, bq, Wk, bk, Wv, bv, Wo, bo):
    query = np.asarray(query, np.float32)
    key = np.asarray(key, np.float32)
    value = np.asarray(value, np.float32)
    Wq, Wk, Wv, Wo = (np.asarray(a, np.float32) for a in (Wq, Wk, Wv, Wo))
    bq, bk, bv = (np.asarray(a, np.float32) for a in (bq, bk, bv))
    masks = _masks()
    vones = np.ones((128, 16, 4, 1), np.float16)
    xb = {}
    for b in range(B):
        xb[b] = (_xblocks(query[b]), _xblocks(key[b]), _xblocks(value[b]))
    in_maps = []
    for c in range(NCORES):
        b, g = divmod(c, 4)
        sl = slice(g * EL, (g + 1) * EL)
        in_maps.append({
            "xqB": xb[b][0],
            "xkB": xb[b][1],
            "xvB": xb[b][2],
            "wqT": np.ascontiguousarray(Wq[sl, :].T).astype(np.float16),
            "wkT": np.ascontiguousarray(Wk[sl, :].T).astype(np.float16),
            "wvT": np.ascontiguousarray(Wv[sl, :].T).astype(np.float16),
            "woT": np.ascontiguousarray(Wo[:, sl].T).astype(np.float16),
            "bq": np.ascontiguousarray(bq[sl]),
            "bk": np.ascontiguousarray(bk[sl]),
            "bv": np.ascontiguousarray(bv[sl]),
            "vones": vones,
            "masks": masks,
        })
    return in_maps


def run(inputs, trace=False, tmpdir=None):
    """Run on 8 cores; returns (full_output, BassKernelResults)."""
    nc = get_nc()
    in_maps = make_in_maps(**inputs)
    res = bass_utils.run_bass_kernel_spmd(
        nc, in_maps, list(range(NCORES)), trace=trace, tmpdir=tmpdir)
    bo = np.asarray(inputs["bo"], np.float32)
    out = np.zeros((B, S, E), np.float32)
    for c in range(NCORES):
        out[c // 4] += res.results[c]["out"]
    out += bo[None, None, :]
    return out, res


def kernel(**inputs):
    out, _ = run(inputs)
    return out


# revision 11
# speedup vs baseline: 1.4016x; 1.4016x over previous
"""Multi-head causal attention (B=2, S=2048, E=1024, H=16, D=64) on 8 TRN2
NeuronCores.

Sharding (data + tensor parallel, Megatron-style):
  core c -> batch b = c // 4, head group g = c % 4 (4 heads, e' = 256 cols).
  Wq/Wk/Wv column-sharded ([256, 1024] slices), Wo row-sharded
  ([1024, 256] slice); each core produces a partial output [2048, 1024]
  (f16) which the host sums per batch group (the Megatron all-reduce) and
  adds bo.

Per-core device kernel (matmul operands fp16, accumulate fp32 in PSUM).
v2 schedule: fine-grained deadline-driven weave.
  - DMA order puts the exp-critical path first (wk, xk0, wq, xq3) so the
    ACT exp stream starts ~21us in; everything else chases.
  - logits pairs (2 heads on PE row-tiles 0-63/64-127) run concurrently;
    attnV (V' ones column -> PSUM row 64 = denominator) lags the exp
    stream; projection/O-proj matmuls are pumped one-at-a-time from a
    deadline-ordered filler queue between attention slots, so the PE
    never idles multi-us on phase boundaries and ACT never starves.
  - normalize: accs copied to SBUF (frees PSUM), reciprocal of the
    denominator row in-place on DVE ([1, 512] rows), gpsimd
    partition-broadcast, DVE multiply into valsT. No DMA transposes.
  - last q-window normalizes directly from PSUM and the tail is just
    4 O-proj tiles + casts + out-DMA.
"""
import sys
import os

sys.path.insert(0, "/opt/trn_rl_repo")

import numpy as np
from contextlib import ExitStack

import concourse.bass as bass  # noqa: E402
import concourse.mybir as mybir  # noqa: E402
import concourse.tile as tile  # noqa: E402
from concourse import bacc, bass_utils  # noqa: E402

bass_utils.upload_artifacts = lambda d: f"local:{d}"

B, S, E, H, D = 2, 2048, 1024, 16, 64
NCORES = 8
EL = 256  # e' columns per core (4 heads)
F32 = mybir.dt.float32
F16 = mybir.dt.float16
AF = mybir.ActivationFunctionType

_CACHE = {}

LAG3 = 6   # attnV lag during (qt=3, c=0): waits for xv DMA + vproj
LAG = 2    # attnV lag elsewhere


def _build():
    nc = bacc.Bacc("TRN2", target_bir_lowering=False, debug=False)

    # x tensors host-pre-blocked: [tb, p, k*512 + m]
    xq_d = nc.dram_tensor("xqB", [4, 128, 8 * 512], F16, kind="ExternalInput")
    xk_d = nc.dram_tensor("xkB", [4, 128, 8 * 512], F16, kind="ExternalInput")
    xv_d = nc.dram_tensor("xvB", [4, 128, 8 * 512], F16, kind="ExternalInput")
    wq_d = nc.dram_tensor("wqT", [E, EL], F16, kind="ExternalInput")
    wk_d = nc.dram_tensor("wkT", [E, EL], F16, kind="ExternalInput")
    wv_d = nc.dram_tensor("wvT", [E, EL], F16, kind="ExternalInput")
    wo_d = nc.dram_tensor("woT", [EL, E], F16, kind="ExternalInput")
    bq_d = nc.dram_tensor("bq", [EL], F32, kind="ExternalInput")
    bk_d = nc.dram_tensor("bk", [EL], F32, kind="ExternalInput")
    bv_d = nc.dram_tensor("bv", [EL], F32, kind="ExternalInput")
    vones_d = nc.dram_tensor("vones", [128, 16, 4, 1], F16, kind="ExternalInput")
    mask_d = nc.dram_tensor("masks", [128, 2, 128], F16, kind="ExternalInput")
    out_d = nc.dram_tensor("out", [S, E], F16, kind="ExternalOutput")

    with tile.TileContext(nc) as tc, ExitStack() as ctx:
        cpool = ctx.enter_context(tc.tile_pool(name="const", bufs=1))
        psp = ctx.enter_context(tc.tile_pool(name="psp", bufs=2, space="PSUM"))
        expp = ctx.enter_context(tc.tile_pool(name="expp", bufs=10))
        opool = ctx.enter_context(tc.tile_pool(name="op", bufs=2))
        smp = ctx.enter_context(tc.tile_pool(name="smp", bufs=2))

        xk = cpool.tile([128, 8, S], F16, tag="xk")
        xq = cpool.tile([128, 8, S], F16, tag="xq")
        xv = cpool.tile([128, 8, S], F16, tag="xv")

        def xblock(x_t, x_d, tb):
            nc.sync.dma_start(
                x_t[:, :, tb * 512:(tb + 1) * 512],
                x_d.ap()[tb].rearrange("p (k m) -> p k m", k=8))

        # ---- DMA issue order == arrival order (single HWDGE ring).
        # Critical path to first exp: wk, xk0, wq, xq3. Then k/v/q blocks
        # by first-use deadline.
        wk = cpool.tile([128, 8, EL], F16, tag="wk")
        nc.sync.dma_start(wk[:], wk_d.ap().rearrange("(k p) m -> p k m", p=128))
        xblock(xk, xk_d, 0)
        wq = cpool.tile([128, 8, EL], F16, tag="wq")
        nc.sync.dma_start(wq[:], wq_d.ap().rearrange("(k p) m -> p k m", p=128))
        xblock(xq, xq_d, 3)
        bkt = cpool.tile([128, 2], F32, tag="bkt")
        nc.sync.dma_start(bkt[:], bk_d.ap().rearrange("(c p) -> p c", p=128))
        bqt = cpool.tile([128, 2], F32, tag="bqt")
        nc.sync.dma_start(bqt[:], bq_d.ap().rearrange("(c p) -> p c", p=128))
        mk2 = cpool.tile([128, 2, 128], F16, tag="mk2")
        nc.sync.dma_start(mk2[:], mask_d.ap())
        xblock(xk, xk_d, 1)
        wv = cpool.tile([128, 8, EL], F16, tag="wv")
        nc.sync.dma_start(wv[:], wv_d.ap().rearrange("(k p) m -> p k m", p=128))
        bvr = cpool.tile([1, EL], F32, tag="bvr")
        nc.sync.dma_start(bvr[:], bv_d.ap().rearrange("(p m) -> p m", p=1))
        bvb = cpool.tile([128, EL], F32, tag="bvb")
        nc.gpsimd.partition_broadcast(bvb[:], bvr[:])
        VP = cpool.tile([128, 16, 4 * 66], F16, tag="VP")  # 66: 4B-aligned
        nc.sync.dma_start(
            VP[:].rearrange("p k (h x) -> p k h x", h=4)[:, :, :, 64:65],
            vones_d.ap())
        xblock(xv, xv_d, 0)
        xblock(xk, xk_d, 2)
        xblock(xv, xv_d, 1)
        xblock(xk, xk_d, 3)
        xblock(xq, xq_d, 2)
        xblock(xv, xv_d, 2)
        xblock(xv, xv_d, 3)
        wo = cpool.tile([128, 2, E], F16, tag="wo")
        nc.sync.dma_start(wo[:], wo_d.ap().rearrange("(c p) m -> p c m", p=128))
        xblock(xq, xq_d, 1)
        xblock(xq, xq_d, 0)

        KT = cpool.tile([128, 2, S], F16, tag="KT")
        QT = cpool.tile([128, 2, S], F16, tag="QT")
        valsT = cpool.tile([128, 2, S], F16, tag="valsT")

        # ---- filler chains: generators yielding after each matmul ----
        def g_kproj(tb, tag="ops"):
            for c in range(2):
                ps = psp.tile([128, 512], F32, tag=tag,
                              bufs=2, name=f"kps{tb}_{c}")
                for k in range(8):
                    nc.tensor.matmul(
                        ps[:],
                        lhsT=wk[:, k, c * 128:(c + 1) * 128],
                        rhs=xk[:, k, tb * 512:(tb + 1) * 512],
                        start=(k == 0), stop=(k == 7))
                    yield
                nc.vector.tensor_scalar_add(
                    KT[:, c, tb * 512:(tb + 1) * 512], ps[:], bkt[:, c:c + 1])

        def g_qproj(tt, tag="ops"):
            for c in range(2):
                ps = psp.tile([128, 512], F32, tag=tag,
                              bufs=2, name=f"qps{tt}_{c}")
                for k in range(8):
                    nc.tensor.matmul(
                        ps[:],
                        lhsT=wq[:, k, c * 128:(c + 1) * 128],
                        rhs=xq[:, k, tt * 512:(tt + 1) * 512],
                        start=(k == 0), stop=(k == 7))
                    yield
                nc.vector.tensor_scalar_add(
                    QT[:, c, tt * 512:(tt + 1) * 512], ps[:], bqt[:, c:c + 1])

        def g_vproj(t3):
            ps = psp.tile([128, EL], F32, tag="ops", bufs=2, name=f"vps{t3}")
            for k in range(8):
                nc.tensor.matmul(
                    ps[:],
                    lhsT=xv[:, k, t3 * 128:(t3 + 1) * 128],
                    rhs=wv[:, k, :],
                    start=(k == 0), stop=(k == 7))
                yield
            nc.vector.tensor_add(
                VP[:, t3, :].rearrange("p (h x) -> p h x", h=4)[:, :, 0:64],
                ps[:].rearrange("p (h x) -> p h x", h=4),
                bvb[:].rearrange("p (h x) -> p h x", h=4))

        def g_oproj(tp):
            # one chain covers tile pair (2*tp, 2*tp+1)
            for ti in range(2):
                tt = 2 * tp + ti
                ot = opool.tile([128, 2, 512], F16, tag="ot", name=f"ot{tt}")
                for eo in range(2):
                    ps = psp.tile([128, 512], F32, tag="ops", bufs=2,
                                  name=f"ops{tt}_{eo}")
                    for c in range(2):
                        nc.tensor.matmul(
                            ps[:],
                            lhsT=valsT[:, c, tt * 128:(tt + 1) * 128],
                            rhs=wo[:, c, eo * 512:(eo + 1) * 512],
                            start=(c == 0), stop=(c == 1))
                        yield
                    nc.vector.tensor_copy(ot[:, eo, :], ps[:])
                nc.sync.dma_start(
                    out_d.ap()[tt * 128:(tt + 1) * 128, :],
                    ot[:].rearrange("p a b -> p (a b)"))

        chains = {}
        gates = {}
        order = []
        slot = [0]  # global lg_exp counter, for DMA-arrival gating

        def add_chain(name, gen, gate=0):
            chains[name] = gen
            gates[name] = gate
            order.append(name)

        def pump(n, force=False):
            # strictly front-of-queue: at most one chain mid-flight, so the
            # shared "ops" PSUM tag never has two incomplete accumulations.
            # A gated front chain (its DMA inputs not yet landed) stops the
            # pump — issuing it would stall the in-order PE queue.
            done = 0
            while done < n and order:
                name = order[0]
                if not force and gates[name] > slot[0]:
                    return
                try:
                    next(chains[name])
                    done += 1
                except StopIteration:
                    del chains[name]
                    order.pop(0)

        def ensure(name):
            while name in chains:
                pump(1, force=True)

        # pre-exp critical path: K block 0 + Q tile 3 (first lg needs only
        # these); kproj(1..3)/vproj/qproj chase as filler.
        for _ in g_kproj(0, tag="lg"):
            pass
        for _ in g_qproj(3, tag="lg"):
            pass

        # deadline-ordered filler for qt=3 c=0: kproj by lg-tile need,
        # vproj by attnV(lag) need. Gates = earliest slot at which the
        # chain's DMA inputs have landed (est. from the single-ring order).
        add_chain("k1", g_kproj(1))
        add_chain("v0", g_vproj(0))
        add_chain("v1", g_vproj(1))
        add_chain("k2", g_kproj(2), gate=1)
        add_chain("v2", g_vproj(2))
        add_chain("v3", g_vproj(3))
        add_chain("v4", g_vproj(4), gate=4)
        add_chain("v5", g_vproj(5), gate=4)
        add_chain("k3", g_kproj(3), gate=6)
        for t in range(6, 16):
            add_chain(f"v{t}", g_vproj(t),
                      gate=(4 if t < 8 else 9 if t < 12 else 11))

        # filler arriving later (registered at the phase that precedes
        # their deadline): qproj(2) during qt3c1, oproj(12..15)+qproj(1)
        # during qt2, oproj(8..11)+qproj(0) during qt1, oproj(4..7) during
        # qt0, oproj(0..3) at the tail.
        late = {
            (3, 1): [("q2", lambda: g_qproj(2))],
            (2, 0): [("o6", lambda: g_oproj(6)), ("o7", lambda: g_oproj(7)),
                     ("q1", lambda: g_qproj(1))],
            (1, 0): [("o4", lambda: g_oproj(4)), ("o5", lambda: g_oproj(5)),
                     ("q0", lambda: g_qproj(0))],
            (0, 0): [("o2", lambda: g_oproj(2)), ("o3", lambda: g_oproj(3))],
        }
        # per-(qt, c) pump budget (filler matmuls issued after each lg_exp)
        budget = {
            (3, 0): 8, (3, 1): 3,
            (2, 0): 4, (2, 1): 2,
            (1, 0): 5, (1, 1): 2,
            (0, 0): 6, (0, 1): 2,
        }

        for qt in range(3, -1, -1):
            nkt = 4 * qt + 4
            lag = LAG3 if qt == 3 else LAG
            accs = {}
            for c in range(2):
                for hh in range(2):
                    accs[(c, hh)] = psp.tile([65, 512], F32, tag="acc",
                                             bufs=2, name=f"acc{qt}_{c}_{hh}")
            exs = {}

            def lg_exp(c, kt):
                dd = kt * 128 - qt * 512
                s = max(dd, 0)
                lg = psp.tile([128, 2, 512], F32, tag="lg", bufs=2,
                              name=f"lg{qt}_{c}_{kt}")
                for hh in range(2):
                    nc.tensor.matmul(
                        lg[:, hh, s:512],
                        lhsT=KT[hh * 64:(hh + 1) * 64, c,
                                kt * 128:(kt + 1) * 128],
                        rhs=QT[hh * 64:(hh + 1) * 64, c,
                               qt * 512 + s:(qt + 1) * 512],
                        start=True, stop=True)
                ex = expp.tile([128, 2, 512], F16, tag="ex",
                               name=f"ex{qt}_{c}_{kt}")
                nc.scalar.activation(ex[:, :, s:512], lg[:, :, s:512], AF.Exp,
                                     scale=0.125)
                if dd >= 0:  # diagonal tile: lower-tri mask on first 128 cols
                    nc.vector.tensor_mul(ex[:, :, s:s + 128],
                                         ex[:, :, s:s + 128], mk2[:])
                exs[(c, kt)] = ex

            def attn_v(c, kt):
                ex = exs.pop((c, kt))
                s = max(kt * 128 - qt * 512, 0)
                for hh in range(2):
                    h = 2 * c + hh
                    nc.tensor.matmul(
                        accs[(c, hh)][:, s:512],
                        lhsT=VP[:, kt, h * 66:h * 66 + 65],
                        rhs=ex[:, hh, s:512],
                        start=(kt == 0), stop=(kt == nkt - 1),
                        skip_group_check=True)

            for c in range(2):
                for name, mk in late.get((qt, c), []):
                    add_chain(name, mk())
                bgt = budget[(qt, c)]
                for kt in range(nkt):
                    if qt == 3 and c == 0 and kt >= 4 and kt % 4 == 0:
                        ensure(f"k{kt // 4}")
                    lg_exp(c, kt)
                    slot[0] += 1
                    pump(bgt)
                    if kt >= lag:
                        if qt == 3 and c == 0:
                            ensure(f"v{kt - lag}")
                        attn_v(c, kt - lag)
                for kt in range(nkt - lag, nkt):
                    if qt == 3 and c == 0:
                        ensure(f"v{kt}")
                    attn_v(c, kt)

                # ---- normalize chunk c ----
                # qt>0: acc -> SBUF copy (frees PSUM), reciprocal of the
                # denominator row in place on DVE, gpsimd broadcast, DVE
                # multiply into valsT. qt==0: straight from PSUM (no reuse
                # pressure at the end).
                svs = {}
                for hh in range(2):
                    sv = smp.tile([65, 512], F32, tag="sv", bufs=4,
                                  name=f"sv{qt}_{c}_{hh}")
                    nc.vector.tensor_copy(sv[:], accs[(c, hh)][:])
                    svs[hh] = sv
                body = {hh: svs[hh][0:64, :] for hh in range(2)}
                # denominator rows DMA-transposed to [128, 4] per hh so the
                # reciprocal runs partition-parallel (4 elems/lane)
                lcol = smp.tile([128, 8], F32, tag="lcol", name=f"lc{qt}_{c}")
                for hh in range(2):
                    nc.sync.dma_start(
                        lcol[:, hh * 4:(hh + 1) * 4],
                        svs[hh][64:65, :].rearrange("p (a b) -> p a b", a=128))
                rcol = smp.tile([128, 8], F32, tag="rcol", name=f"rc{qt}_{c}")
                nc.vector.reciprocal(rcol[:], lcol[:])
                rrow = smp.tile([1, 2, 512], F32, tag="rrow", bufs=2,
                                name=f"rr{qt}_{c}")
                for hh in range(2):
                    nc.sync.dma_start(
                        rrow[0:1, hh, :].rearrange("p (a b) -> p a b", a=128),
                        rcol[:, hh * 4:(hh + 1) * 4])
                for hh in range(2):
                    bc = smp.tile([128, 512], F32, tag="bc", bufs=4,
                                  name=f"bc{qt}_{c}_{hh}")
                    nc.gpsimd.partition_broadcast(bc[0:64, :], rrow[0:1, hh, :])
                    nc.vector.tensor_mul(
                        valsT[hh * 64:(hh + 1) * 64, c,
                              qt * 512:(qt + 1) * 512],
                        body[hh], bc[0:64, :])

        # tail: first q-window's O-projection pairs
        for name in ("o0", "o1"):
            add_chain(name, g_oproj(int(name[1])))
        while order:
            pump(1000)

    nc.compile()
    return nc


def get_nc():
    if "nc" not in _CACHE:
        _CACHE["nc"] = _build()
    return _CACHE["nc"]


def _masks():
    i = np.arange(128)[:, None]
    j = np.arange(128)[None, :]
    m = (i <= j).astype(np.float16)  # within-window causal: keep k <= q
    return np.broadcast_to(m[:, None, :], (128, 2, 128)).copy()


def _xblocks(x):
    # [S, E] f32 -> [4, 128, 8*512] f16: blk[tb, p, k*512+m] = x[tb*512+m, k*128+p]
    xT = np.ascontiguousarray(x.T).astype(np.float16)  # [E, S]
    return np.ascontiguousarray(
        xT.reshape(8, 128, 4, 512).transpose(2, 1, 0, 3).reshape(4, 128, 4096))


def make_in_maps(query, key, value, Wq, bq, Wk, bk, Wv, bv, Wo, bo):
    query = np.asarray(query, np.float32)
    key = np.asarray(key, np.float32)
    value = np.asarray(value, np.float32)
    Wq, Wk, Wv, Wo = (np.asarray(a, np.float32) for a in (Wq, Wk, Wv, Wo))
    bq, bk, bv = (np.asarray(a, np.float32) for a in (bq, bk, bv))
    masks = _masks()
    vones = np.ones((128, 16, 4, 1), np.float16)
    xb = {}
    for b in range(B):
        xb[b] = (_xblocks(query[b]), _xblocks(key[b]), _xblocks(value[b]))
    in_maps = []
    for c in range(NCORES):
        b, g = divmod(c, 4)
        sl = slice(g * EL, (g + 1) * EL)
        in_maps.append({
            "xqB": xb[b][0],
            "xkB": xb[b][1],
            "xvB": xb[b][2],
            "wqT": np.ascontiguousarray(Wq[sl, :].T).astype(np.float16),
            "wkT": np.ascontiguousarray(Wk[sl, :].T).astype(np.float16),
            "wvT": np.ascontiguousarray(Wv[sl, :].T).astype(np.float16),
            "woT": np.ascontiguousarray(Wo[:, sl].T).astype(np.float16),
            "bq": np.ascontiguousarray(bq[sl]),
            "bk": np.ascontiguousarray(bk[sl]),
            "bv": np.ascontiguousarray(bv[sl]),
            "vones": vones,
            "masks": masks,
        })
    return in_maps


def run(inputs, trace=False, tmpdir=None):
    """Run on 8 cores; returns (full_output, BassKernelResults)."""
    nc = get_nc()
    in_maps = make_in_maps(**inputs)
    res = bass_utils.run_bass_kernel_spmd(
        nc, in_maps, list(range(NCORES)), trace=trace, tmpdir=tmpdir)
    bo = np.asarray(inputs["bo"], np.float32)
    out = np.zeros((B, S, E), np.float32)
    for c in range(NCORES):
        out[c // 4] += res.results[c]["out"]
    out += bo[None, None, :]
    return out, res


def kernel(**inputs):
    out, _ = run(inputs)
    return out
